# revision 39
# baseline (speedup 1.0000x reference)
"""Trainium2 Bass kernel for ChannelCrissCrossAttention (v3).

Shapes (hardcoded): B=8, IN=128, C=16, V=T=64.
Sharding: pure data parallel, one batch element per NeuronCore (8 cores).

Math (same as v2): per batch element
  q,k,v = conv3x3(x; wq/wk/wv) -> [C, V, T].  Row grids r=(c,a):
  G_r[x,j] = exp(q[c,a,x]*k[c,a,j]) serve tt (natural) and vv (spatial
  transpose).  cc grids live on 16-elem strips of the flat channel-major
  layout at spatially-transposed positions.  Z = S_tt + S_vv + S_cc;
  outputs W_*/Z; stacked reverse conv; gamma*out + x.

v3 vs v2:
 - bf16 matmuls (4x PE throughput vs fp32) and bf16 DVE element-wise ops
   (2x_1p perf mode).
 - No DRAM scratch at all: every spread/rearrange is an SBUF->SBUF DMA.
 - S/W row-sums via fold-trees (bf16 tensor_add at 2x) + short f32 reduce.
 - Outer-product muls (the only 1x DVE pass) offloaded to the Pool/GpSimd
   engine so DVE, Pool, ACT and PE all run concurrently.
 - Batched combine (Z, 1/Z, three weighted outputs) after the b-loop.
"""

import sys

sys.path.insert(0, "/opt/trn_rl_repo")

import numpy as np
import ml_dtypes

import concourse.bass as bass
import concourse.tile as tile
from concourse import bacc, mybir
from concourse.bass_utils import run_bass_kernel_spmd

F32 = mybir.dt.float32
BF16 = mybir.dt.bfloat16
AF = mybir.ActivationFunctionType
ALU = mybir.AluOpType
AX = mybir.AxisListType

IN, C, V, T = 128, 16, 64, 64
CH3 = 3 * C  # 48
NPOS = V * T  # 4096
PW = 66  # padded spatial width


def _build_program(niter=1):
    nc = bacc.Bacc("TRN2", target_bir_lowering=False, debug=False)

    x_d = nc.dram_tensor("x", [IN, V, T], BF16, kind="ExternalInput")
    wqkv_d = nc.dram_tensor("wqkv", [IN, 9 * CH3], BF16, kind="ExternalInput")
    bqkv_d = nc.dram_tensor("bqkv", [CH3, 1], F32, kind="ExternalInput")
    wr_d = nc.dram_tensor("wr", [CH3, 9 * IN], BF16, kind="ExternalInput")
    gb_d = nc.dram_tensor("gb", [IN, 1], F32, kind="ExternalInput")
    id_d = nc.dram_tensor("ident", [128, 128], F32, kind="ExternalInput")
    out_d = nc.dram_tensor("out", [IN, V, T], F32, kind="ExternalOutput")
    qkv_h = nc.dram_tensor("qkv_h", [CH3, V, T], BF16)
    st_h = nc.dram_tensor("st_h", [2 * C, V, T], BF16)
    ocat_h = nc.dram_tensor("ocat_h", [CH3, V, T], BF16)

    with tile.TileContext(nc) as tc:
        if niter == 1:
            _body(nc, tc, x_d, wqkv_d, bqkv_d, wr_d, gb_d, id_d, out_d, qkv_h,
                  st_h, ocat_h)
        else:
            with tc.For_i(0, niter, 1):
                _body(nc, tc, x_d, wqkv_d, bqkv_d, wr_d, gb_d, id_d, out_d,
                      qkv_h, st_h, ocat_h)

    nc.compile()
    return nc


def _body(nc, tc, x_d, wqkv_d, bqkv_d, wr_d, gb_d, id_d, out_d, qkv_h, st_h,
          ocat_h):
    from contextlib import ExitStack
    ctx = ExitStack()
    persist = ctx.enter_context(tc.tile_pool(name="persist", bufs=1))
    pio = ctx.enter_context(tc.tile_pool(name="pio", bufs=2))
    pP = ctx.enter_context(tc.tile_pool(name="pP", bufs=2))
    pG = ctx.enter_context(tc.tile_pool(name="pG", bufs=2))
    pF = ctx.enter_context(tc.tile_pool(name="pF", bufs=1))
    pcomb = ctx.enter_context(tc.tile_pool(name="pcomb", bufs=1))
    psum1 = ctx.enter_context(
        tc.tile_pool(name="psum1", bufs=2, space=bass.MemorySpace.PSUM))
    psum2 = ctx.enter_context(
        tc.tile_pool(name="psum2", bufs=2, space=bass.MemorySpace.PSUM))
    psumS = ctx.enter_context(
        tc.tile_pool(name="psumS", bufs=1, space=bass.MemorySpace.PSUM))

    # ---- Phase 0: weights + padded input ----
    wqkv = persist.tile([IN, 9 * CH3], BF16)
    nc.sync.dma_start(wqkv[:], wqkv_d.ap())
    bqkv = persist.tile([CH3, 1], F32)
    nc.sync.dma_start(bqkv[:], bqkv_d.ap())
    wr = persist.tile([CH3, 9 * IN], BF16)
    nc.sync.dma_start(wr[:], wr_d.ap())
    gb = persist.tile([IN, 1], F32)
    nc.sync.dma_start(gb[:], gb_d.ap())
    ident = persist.tile([128, 128], F32)
    nc.sync.dma_start(ident[:], id_d.ap())

    xpad = persist.tile([IN, PW * PW], BF16)
    xpad_v = xpad[:].rearrange("p (v t) -> p v t", v=PW)
    # zero only the 1-wide border (top/bottom rows, left/right cols)
    nc.gpsimd.memset(xpad_v[:, 0, :], 0.0)
    nc.gpsimd.memset(xpad_v[:, PW - 1, :], 0.0)
    nc.gpsimd.memset(xpad_v[:, 1:PW - 1, 0:1], 0.0)
    nc.gpsimd.memset(xpad_v[:, 1:PW - 1, PW - 1:PW], 0.0)
    for m in range(8):
        nc.sync.dma_start(
            xpad_v[:, 1 + m * 8:1 + (m + 1) * 8, 1:1 + T],
            x_d.ap()[:, m * 8:(m + 1) * 8, :])

    opad = persist.tile([CH3, PW * PW], BF16)
    opad_v = opad[:].rearrange("p (v t) -> p v t", v=PW)
    nc.gpsimd.memset(opad_v[:, 0, :], 0.0)
    nc.gpsimd.memset(opad_v[:, PW - 1, :], 0.0)
    nc.gpsimd.memset(opad_v[:, 1:PW - 1, 0:1], 0.0)
    nc.gpsimd.memset(opad_v[:, 1:PW - 1, PW - 1:PW], 0.0)

    # ---- Phase 1: qkv conv -> qkv (SBUF, bf16, [48, v, t]) ----
    qkv = persist.tile([CH3, V, T], BF16)
    for m in range(8):
        ps = psum1.tile([IN, 512], F32, tag="mm")
        for tap in range(9):
            dy, dx = tap // 3, tap % 3
            rhs = xpad_v[:, m * 8 + dy: m * 8 + dy + 8, dx: dx + T]
            nc.tensor.matmul(
                ps[0:CH3, :], wqkv[:, tap * CH3:(tap + 1) * CH3], rhs,
                start=(tap == 0), stop=(tap == 8))
        nc.scalar.activation(
            qkv[:, m * 8:(m + 1) * 8, :].rearrange("p v t -> p (v t)"),
            ps[0:CH3, :], AF.Identity, bias=bqkv[:])
        # mirror to DRAM for the cc strip gathers (flat-layout source)
        nc.sync.dma_start(qkv_h.ap()[:, m * 8:(m + 1) * 8, :],
                          qkv[:, m * 8:(m + 1) * 8, :])

    # ---- Phase 2: spreads, all SBUF->SBUF ----
    # Branch A operands: [(c2 v), b, t] <- qkv[2b+c2, v, t]
    QA = persist.tile([128, 8, T], BF16)
    KA = persist.tile([128, 8, T], BF16)
    VA = persist.tile([128, 8, T], BF16)
    for i, dst in enumerate((QA, KA, VA)):
        src = qkv_h.ap()[i * C:(i + 1) * C]  # [16, 64, 64] (DRAM mirror)
        for c2 in range(2):
            nc.sync.dma_start(
                dst[c2 * 64:(c2 + 1) * 64, :, :],
                src.rearrange("(b c2) v t -> c2 v b t", c2=2)[c2])

    # cc operands, slot pi = a*64+b -> partition P = (a%2)*64+b, chunk ah=a//2.
    # q/k strips at spatially transposed positions: flat offset within the
    # 16-channel block = 4096*(b//4) + 1024*(b%4) + 32*ah + 16*al + j,
    # i.e. channel p=b//4, free (bl ah al j) contiguous split (4, 32, 2, 16).
    q_cc = persist.tile([128, 32, C], BF16)
    k_cc = persist.tile([128, 32, C], BF16)
    v_cc = persist.tile([128, 32, C], BF16)
    for i, dst in ((0, q_cc), (1, k_cc)):
        flat = qkv_h.ap()[i * C:(i + 1) * C].flatten()  # [65536]
        srcv = flat.rearrange("(p64 ah al j) -> al p64 ah j",
                              p64=64, al=2, ah=32, j=C)
        for al in range(2):
            nc.sync.dma_start(dst[al * 64:(al + 1) * 64, :, :], srcv[al])
    # v strips natural: offset 16*pi = 2048*ah + 1024*al + 16*b
    vflat = qkv_h.ap()[2 * C:3 * C].flatten()
    vsv = vflat.rearrange("(ah al b j) -> al b ah j", ah=32, al=2, b=64, j=C)
    for al in range(2):
        nc.sync.dma_start(v_cc[al * 64:(al + 1) * 64, :, :], vsv[al])

    # ---- Phase 3: cc compute ----
    # P on Pool, exp on ACT; G and H=G*v interleaved in one tile so each
    # fold/reduce instruction covers both the S and W chains.
    SW_cc = persist.tile([128, 2, 32, C], F32)
    NPH = 8
    for chk in range(32 // NPH):
        sl = slice(chk * NPH, (chk + 1) * NPH)
        qs = q_cc[:, sl, :]
        ks = k_cc[:, sl, :]
        vs = v_cc[:, sl, :]
        Pc = pP.tile([128, NPH, C, C], F32, tag="Pcc")
        nc.gpsimd.tensor_mul(
            Pc[:],
            qs.unsqueeze(3).broadcast_to([128, NPH, C, C]),
            ks.unsqueeze(2).broadcast_to([128, NPH, C, C]))
        GHc = pG.tile([128, 2, NPH, C, C], BF16, tag="GHcc")
        nc.scalar.activation(GHc[:, 0, :, :, :], Pc[:], AF.Exp)
        nc.vector.tensor_mul(
            GHc[:, 1, :, :, :], GHc[:, 0, :, :, :],
            vs.unsqueeze(2).broadcast_to([128, NPH, C, C]))
        GHv = GHc[:].rearrange("p sw a c j -> p sw (a c) j")
        F1 = pF.tile([128, 2, NPH * C, 8], BF16, tag="ccF1")
        nc.vector.tensor_add(F1[:], GHv[:, :, :, 0:8], GHv[:, :, :, 8:16])
        F2 = pF.tile([128, 2, NPH * C, 4], BF16, tag="ccF2")
        nc.vector.tensor_add(F2[:], F1[:, :, :, 0:4], F1[:, :, :, 4:8])
        F3 = pF.tile([128, 2, NPH * C, 2], BF16, tag="ccF3")
        nc.vector.tensor_add(F3[:], F2[:, :, :, 0:2], F2[:, :, :, 2:4])
        nc.vector.tensor_reduce(SW_cc[:, :, sl, :], F3[:], axis=AX.X,
                                op=ALU.add)

    # ---- Phase 4: batched PE back-transpose. One [128,128] transpose per
    # 8-chunk group: out partition (k' j), free P; column ah*128+P of the
    # [16, 4096] channel-major strip-sum layout goes straight to DRAM.
    for ti in (0, 1):
        dstv = st_h.ap()[ti * C:(ti + 1) * C].rearrange(
            "c v t -> c (v t)").rearrange(
            "c (kk kp p) -> kk kp c p", kk=4, kp=8, p=128)
        for k in range(4):
            pt = psum2.tile([128, 128], F32, tag="ps2")
            nc.tensor.matmul(
                pt[:], SW_cc[:, ti, k * 8:(k + 1) * 8, :].rearrange(
                    "p a j -> p (a j)"), ident[:], is_transpose=True)
            st2 = pio.tile([128, 128], BF16, tag="st2")
            nc.scalar.copy(st2[:], pt[:])
            nc.sync.dma_start(dstv[k], st2[:])
    ScA = persist.tile([128, 8, T], BF16)
    WcA = persist.tile([128, 8, T], BF16)
    for i, dst in ((0, ScA), (1, WcA)):
        src = st_h.ap()[i * C:(i + 1) * C]  # [16, 64, 64]
        for c2 in range(2):
            nc.sync.dma_start(
                dst[c2 * 64:(c2 + 1) * 64, :, :],
                src.rearrange("(b c2) v t -> c2 v b t", c2=2)[c2])

    # ---- Phase 5: branch A ----
    # S and W row-sums interleaved per b: [p, b, sw, t]; the vv transposes
    # then handle S and W in one [64, 128] call each, landing S on PSUM
    # partitions 0-63 and W on 64-127 (base partition 0 as required).
    SW_all = persist.tile([128, 8, 2, T], F32)
    SW2P = [psumS.tile([128, 8 * T], F32, tag=f"SW2P{c2}", name=f"SW2P{c2}")
            for c2 in range(2)]

    for b in range(8):
        Qb = QA[:, b, :]
        Kb = KA[:, b, :]
        Vb = VA[:, b, :]

        P = pP.tile([128, T, T], F32, tag="P")
        nc.gpsimd.tensor_mul(
            P[:],
            Qb.unsqueeze(2).broadcast_to([128, T, T]),
            Kb.unsqueeze(1).broadcast_to([128, T, T]))
        GH = pG.tile([128, 2, T, T], BF16, tag="GH")
        nc.scalar.activation(GH[:, 0, :, :], P[:], AF.Exp)
        Sb = SW_all[:, b, 0, :]
        Wb = SW_all[:, b, 1, :]
        nc.vector.tensor_mul(
            GH[:, 1, :, :], GH[:, 0, :, :],
            Vb.unsqueeze(1).broadcast_to([128, T, T]))
        # fused S/W chains: fold 64->32->16, f32 reduce over 16
        F1 = pF.tile([128, 2, T, 32], BF16, tag="F1")
        nc.vector.tensor_add(F1[:], GH[:, :, :, 0:32], GH[:, :, :, 32:64])
        F2 = pF.tile([128, 2, T, C], BF16, tag="F2")
        nc.vector.tensor_add(F2[:], F1[:, :, :, 0:16], F1[:, :, :, 16:32])
        F3 = pF.tile([128, 2, T, 8], BF16, tag="F3")
        nc.vector.tensor_add(F3[:], F2[:, :, :, 0:8], F2[:, :, :, 8:16])
        nc.vector.tensor_reduce(SW_all[:, b, :, :], F3[:], axis=AX.X,
                                op=ALU.add)

        # vv terms: one [64, 128] transpose per c2 covers S and W at once
        for c2 in range(2):
            rows = slice(c2 * 64, (c2 + 1) * 64)
            cols = slice(b * T, (b + 1) * T)
            nc.tensor.matmul(
                SW2P[c2][:, cols],
                SW_all[rows, b, :, :].rearrange("p sw t -> p (sw t)"),
                ident[rows, rows], is_transpose=True)

    # ---- Phase 5b/5c: combine + scatter, split into two b-halves so the
    # first half's DMA overlaps the second half's attention compute ----
    Sf3 = SW_all[:, :, 0, :]
    Wf3 = SW_all[:, :, 1, :]
    Z = pcomb.tile([128, 8 * T], F32, tag="Z")
    R = pcomb.tile([128, 8 * T], F32, tag="R")
    Ot = pcomb.tile([128, 8 * T], BF16, tag="Ot")
    Ov = pcomb.tile([128, 8 * T], BF16, tag="Ov")
    Oc = pcomb.tile([128, 8 * T], BF16, tag="Oc")
    Z3 = Z[:].rearrange("p (b t) -> p b t", b=8)
    R3 = R[:].rearrange("p (b t) -> p b t", b=8)
    Ot3 = Ot[:].rearrange("p (b t) -> p b t", b=8)
    Ov3 = Ov[:].rearrange("p (b t) -> p b t", b=8)
    Oc3 = Oc[:].rearrange("p (b t) -> p b t", b=8)
    SWp3 = [SW2P[c2][:].rearrange("p (b t) -> p b t", b=8) for c2 in range(2)]
    for h in range(2):
        h4 = slice(h * 4, (h + 1) * 4)
        for c2 in range(2):
            rows = slice(c2 * 64, (c2 + 1) * 64)
            nc.vector.tensor_add(Z3[rows, h4, :], Sf3[rows, h4, :],
                                 SWp3[c2][0:64, h4, :])
        nc.gpsimd.tensor_add(Z3[:, h4, :], Z3[:, h4, :], ScA[:, h4, :])
        nc.vector.reciprocal(R3[:, h4, :], Z3[:, h4, :])
        nc.gpsimd.tensor_mul(Ot3[:, h4, :], Wf3[:, h4, :], R3[:, h4, :])
        for c2 in range(2):
            rows = slice(c2 * 64, (c2 + 1) * 64)
            nc.vector.tensor_mul(Ov3[rows, h4, :], SWp3[c2][64:128, h4, :],
                                 R3[rows, h4, :])
        nc.gpsimd.tensor_mul(Oc3[:, h4, :], WcA[:, h4, :], R3[:, h4, :])
        for g, t_ in ((0, Oc), (1, Ov), (2, Ot)):
            tv = t_[:].rearrange("p (b t) -> p b t", b=8)
            dstv = ocat_h.ap()[g * C:(g + 1) * C].rearrange(
                "(b c2) v t -> c2 v b t", c2=2)
            for c2 in range(2):
                nc.sync.dma_start(
                    dstv[c2][:, h * 4:(h + 1) * 4, :],
                    tv[c2 * 64:(c2 + 1) * 64, h * 4:(h + 1) * 4, :])
    # per-chunk gathers into opad so the reverse conv can start early
    for g8 in range(8):
        nc.sync.dma_start(
            opad_v[:, 1 + g8 * 8:1 + (g8 + 1) * 8, 1:1 + T],
            ocat_h.ap()[:, g8 * 8:(g8 + 1) * 8, :])

    # ---- Phase 6: reverse conv + residual ----
    for m in range(8):
        ps2 = psum1.tile([IN, 512], F32, tag="mm")
        for tap in range(9):
            dy, dx = tap // 3, tap % 3
            rhs = opad_v[:, m * 8 + dy: m * 8 + dy + 8, dx: dx + T]
            nc.tensor.matmul(
                ps2[:], wr[:, tap * IN:(tap + 1) * IN], rhs,
                start=(tap == 0), stop=(tap == 8))
        o_sb = pio.tile([IN, 512], F32, tag="o_sb")
        xin = xpad_v[:, m * 8 + 1: m * 8 + 9, 1:1 + T]
        nc.scalar.activation(o_sb[:], ps2[:], AF.Identity, bias=gb[:])
        nc.vector.tensor_add(
            o_sb[:].rearrange("p (v t) -> p v t", v=8),
            o_sb[:].rearrange("p (v t) -> p v t", v=8), xin)
        nc.sync.dma_start(out_d.ap()[:, m * 8:(m + 1) * 8, :],
                          o_sb[:].rearrange("p (v t) -> p v t", v=8))

    ctx.close()


_NC_CACHE = {}


def _get_program(niter=1):
    if niter not in _NC_CACHE:
        _NC_CACHE[niter] = _build_program(niter)
    return _NC_CACHE[niter]


def _host_weights(wq, bq, wk, bk, wv, bv, wcr, bcr, wvr, bvr, wtr, btr, gamma):
    g = np.float32(np.asarray(gamma).reshape(-1)[0])
    wf = np.concatenate([wq, wk, wv], axis=0)  # [48, 128, 3, 3]
    wqkv = np.ascontiguousarray(
        wf.transpose(1, 2, 3, 0).reshape(IN, 9 * CH3)).astype(ml_dtypes.bfloat16)
    bqkv = np.concatenate([bq, bk, bv]).reshape(CH3, 1).astype(np.float32)
    wrf = np.concatenate([wcr, wvr, wtr], axis=1) * g  # [128, 48, 3, 3]
    wr_ = np.ascontiguousarray(
        wrf.transpose(1, 2, 3, 0).reshape(CH3, 9 * IN)).astype(ml_dtypes.bfloat16)
    gb = (g * (bcr + bvr + btr)).reshape(IN, 1).astype(np.float32)
    return wqkv, bqkv, wr_, gb


def kernel(x, wq, bq, wk, bk, wv, bv, wcr, bcr, wvr, bvr, wtr, btr, gamma,
           _trace=False, _niter=1):
    nc = _get_program(_niter)
    wqkv, bqkv, wr_, gb = _host_weights(
        wq, bq, wk, bk, wv, bv, wcr, bcr, wvr, bvr, wtr, btr, gamma)
    x = np.asarray(x, dtype=np.float32).astype(ml_dtypes.bfloat16)
    ident = np.eye(128, dtype=np.float32)
    in_maps = [
        {"x": np.ascontiguousarray(x[i]), "wqkv": wqkv, "bqkv": bqkv,
         "wr": wr_, "gb": gb, "ident": ident}
        for i in range(8)
    ]
    res = run_bass_kernel_spmd(nc, in_maps, list(range(8)), trace=_trace)
    out = np.stack([res.results[i]["out"] for i in range(8)]).astype(np.float32)
    if _trace:
        kernel.last_exec_time_ns = res.exec_time_ns
        kernel.last_results = res
    return out
